# revision 8
# baseline (speedup 1.0000x reference)
"""MemEffEquivariantAttention TRN2 Bass kernel (transposed-scores flow).

Sharding: 8 cores = 4 batches x 2 query-token halves (fully data-parallel,
no collectives).

Key idea vs the previous version: scores are computed TRANSPOSED
(wT[s, t] = kT_chunk.T @ qT), so the attention probabilities already have
s on partitions and feed the attn matmul directly -- the 8.4MB SBUF->SBUF
gpsimd transpose-gather of u is gone entirely.  Z (the softmax denominator,
per (head, t)) is recovered with a ones-vector matmul over the s-partitions,
and 1/Z is applied to the [96, 256] attn output via a gpsimd
partition-broadcast + one DVE multiply per head.

Other changes driven by the DMA-bound trace of the previous version
(DMA active 130us of 155us; 31MB moved):
  - q/k in fp16 (not f32r): halves their traffic; score error ~4e-3 abs.
  - v PBC expansion done on host, packed [128, 8sc, H, 96] for direct
    per-(sc,h) lhsT slices; no device gather.
  - bias: for N_EB heads the host ships exp(bias) and the bias add becomes
    a DVE multiply (exp(w)*exp(b)); for the rest the host ships bias^T and
    it lands in PSUM via one identity matmul per half-head.  N_EB balances
    DVE vs PE load.
  - exp uses a constant -40 bias (softmax shift, folded out exactly by Z)
    to keep e/m0 comfortably in bf16/f32 range.
  - output stored bf16, upcast to f32 on host.
"""
import sys
sys.path.insert(0, "/opt/trn_rl_repo")

import numpy as np
import ml_dtypes

import concourse.bacc as bacc
import concourse.tile as tile
from concourse import mybir
from concourse.bass_utils import run_bass_kernel_spmd

F32 = mybir.dt.float32
F16 = mybir.dt.float16
BF16 = mybir.dt.bfloat16
AF = mybir.ActivationFunctionType

B, T, P, HID = 4, 512, 3, 512
HD, H = 32, 16
EXP, S = 512, 1024
TQ = 256            # query tokens per core
EPS = 1e-3
CUTOFF = 1e-5
NEG = -1e30
D = P * HD          # 96, per-head feature dim
SHIFT = -40.0       # constant softmax shift, cancels exactly via Z
N_EB = 8            # heads 0..N_EB-1 use host exp(bias); rest PE-identity

_prog_cache = {}


def _build_program():
    nc = bacc.Bacc("TRN2", target_bir_lowering=False, debug=False)

    qT_d = nc.dram_tensor("qT", [H, D, TQ], F16, kind="ExternalInput").ap()
    kT_d = nc.dram_tensor("kT", [H, D, S], F16, kind="ExternalInput").ap()
    vS_d = nc.dram_tensor("vS", [128, 8, H, D], BF16, kind="ExternalInput").ap()
    # bt: heads < N_EB hold exp(bias)^T, heads >= N_EB hold bias^T
    bt_d = nc.dram_tensor("bt", [H, 128, 2, 4 * TQ], BF16, kind="ExternalInput").ap()
    lawT_d = nc.dram_tensor("lawT", [128, 2, 4 * TQ], BF16, kind="ExternalInput").ap()
    WT_d = nc.dram_tensor("WT", [128, 4, HID], BF16, kind="ExternalInput").ap()
    eye_d = nc.dram_tensor("eye128", [128, 128], BF16, kind="ExternalInput").ap()
    ones128_d = nc.dram_tensor("ones128", [128, 1], BF16, kind="ExternalInput").ap()
    ones96_d = nc.dram_tensor("ones96", [D, 1], F32, kind="ExternalInput").ap()
    out_d = nc.dram_tensor("out", [TQ, P, HID], BF16, kind="ExternalOutput").ap()

    with tile.TileContext(nc) as tc:
        with tc.tile_pool(name="const", bufs=1) as cp, \
             tc.tile_pool(name="kq", bufs=3) as kq, \
             tc.tile_pool(name="btp", bufs=3) as btp, \
             tc.tile_pool(name="eu", bufs=2) as eu, \
             tc.tile_pool(name="work", bufs=3) as wp, \
             tc.tile_pool(name="psw", bufs=2, space="PSUM") as psw, \
             tc.tile_pool(name="psa", bufs=2, space="PSUM") as psa, \
             tc.tile_pool(name="psz", bufs=2, space="PSUM") as psz:

            # ---- constants / preload ----
            vS_t = cp.tile([128, 8, H, D], BF16, tag="vS")
            lawT_t = cp.tile([128, 2, 4 * TQ], BF16, tag="lawT")
            WT_t = cp.tile([128, 4, HID], BF16, tag="WT")
            eye_t = cp.tile([128, 128], BF16, tag="eye")
            ones128_t = cp.tile([128, 1], BF16, tag="o128")
            ones96_t = cp.tile([D, 1], F32, tag="o96")
            eps_t = cp.tile([128, 1], F32, tag="eps")
            shift_t = cp.tile([128, 1], F32, tag="shift")
            X_t = cp.tile([128, P, 4, TQ], BF16, tag="X")
            sqacc_t = cp.tile([D, TQ], F32, tag="sqacc")
            nc.vector.memset(eps_t[:], EPS)
            nc.vector.memset(shift_t[:], SHIFT)

            nc.sync.dma_start(out=lawT_t[:], in_=lawT_d)
            nc.scalar.dma_start(out=eye_t[:], in_=eye_d)
            nc.scalar.dma_start(out=ones128_t[:], in_=ones128_d)
            nc.scalar.dma_start(out=ones96_t[:], in_=ones96_d)

            kT_tiles, qT_tiles, bt_tiles = {}, {}, {}
            m0_tiles, u_tiles, z_tiles, at_tiles = {}, {}, {}, {}

            def emit_loads(h):
                kT_t = kq.tile([D, S], F16, tag="kT", name=f"kT_{h}")
                qT_t = kq.tile([D, TQ], F16, tag="qT", name=f"qT_{h}")
                bt_t = btp.tile([128, 2, 4 * TQ], BF16, tag="bt", name=f"bt_{h}")
                nc.sync.dma_start(out=kT_t[:], in_=kT_d[h])
                nc.scalar.dma_start(out=qT_t[:], in_=qT_d[h])
                nc.scalar.dma_start(out=bt_t[:], in_=bt_d[h])
                kT_tiles[h], qT_tiles[h], bt_tiles[h] = kT_t, qT_t, bt_t

            def emit_scores(h):
                kT_t = kT_tiles.pop(h)
                qT_t = qT_tiles.pop(h)
                bt_t = bt_tiles.pop(h)
                e_t = eu.tile([128, 2, 4 * TQ], BF16, tag="e", name=f"e_{h}")
                m0_t = e_t if h >= N_EB else eu.tile([128, 2, 4 * TQ], BF16,
                                                     tag="m0", name=f"m0_{h}")
                u_t = eu.tile([128, 2, 4 * TQ], BF16, tag="u", name=f"u_{h}")
                for hf in range(2):
                    w_ps = psw.tile([128, 4 * TQ], F32, tag="w",
                                    name=f"w_{h}_{hf}")
                    if h >= N_EB:
                        # bias lands in PSUM via identity matmul
                        # (one matmul per PSUM bank: out <= 512 fp32)
                        for bk in range(2):
                            bs = slice(bk * 512, (bk + 1) * 512)
                            nc.tensor.matmul(w_ps[:, bs], eye_t[:],
                                             bt_t[:, hf, bs],
                                             start=True, stop=False,
                                             skip_group_check=True)
                    for sc4 in range(4):
                        sc = 4 * hf + sc4
                        nc.tensor.matmul(w_ps[:, sc4 * TQ:(sc4 + 1) * TQ],
                                         kT_t[:, sc * 128:(sc + 1) * 128],
                                         qT_t[:],
                                         start=(h < N_EB), stop=True,
                                         skip_group_check=True)
                    nc.scalar.activation(e_t[:, hf, :], w_ps[:], AF.Exp,
                                         bias=shift_t[:])
                    if h < N_EB:
                        nc.vector.tensor_mul(m0_t[:, hf, :], e_t[:, hf, :],
                                             bt_t[:, hf, :])
                    nc.vector.tensor_mul(u_t[:, hf, :], m0_t[:, hf, :],
                                         lawT_t[:, hf, :])
                m0_tiles[h], u_tiles[h] = m0_t, u_t

            def emit_post_pe(h):
                m0_t, u_t = m0_tiles[h], u_tiles[h]
                z_ps = psz.tile([1, TQ], F32, tag="z", name=f"z_{h}")
                for sc in range(8):
                    nc.tensor.matmul(z_ps[:], ones128_t[:],
                                     m0_t[:, sc // 4,
                                          (sc % 4) * TQ:(sc % 4 + 1) * TQ],
                                     start=(sc == 0), stop=(sc == 7))
                at_ps = psa.tile([D, TQ], F32, tag="at", name=f"at_{h}")
                for sc in range(8):
                    nc.tensor.matmul(at_ps[:], vS_t[:, sc, h, :],
                                     u_t[:, sc // 4,
                                         (sc % 4) * TQ:(sc % 4 + 1) * TQ],
                                     start=(sc == 0), stop=(sc == 7))
                z_tiles[h], at_tiles[h] = z_ps, at_ps

            def emit_post_dve(h):
                del m0_tiles[h], u_tiles[h]
                at_ps = at_tiles.pop(h)
                rz_t = wp.tile([1, TQ], F32, tag="rz", name=f"rz_{h}")
                nc.vector.reciprocal(rz_t[:], z_tiles.pop(h)[:])
                rzb_t = wp.tile([D, TQ], F32, tag="rzb", name=f"rzb_{h}")
                nc.gpsimd.partition_broadcast(rzb_t[:], rz_t[:])
                at_sb = wp.tile([D, TQ], BF16, tag="atsb", name=f"atsb_{h}")
                nc.vector.tensor_mul(at_sb[:], at_ps[:], rzb_t[:])
                # stash into X[(h%4)*32+j, p, h//4, t] for out_proj lhsT
                for p in range(P):
                    nc.gpsimd.dma_start(
                        out=X_t[(h % 4) * 32:(h % 4 + 1) * 32, p, h // 4, :],
                        in_=at_sb[p * 32:(p + 1) * 32, :])
                # sumsq accumulate on DVE (f32 accumulator)
                if h == 0:
                    nc.vector.tensor_mul(sqacc_t[:], at_sb[:], at_sb[:])
                else:
                    sq_t = wp.tile([D, TQ], BF16, tag="sq")
                    nc.vector.tensor_mul(sq_t[:], at_sb[:], at_sb[:])
                    nc.vector.tensor_add(sqacc_t[:], sqacc_t[:], sq_t[:])

            # ---- main loop, software-pipelined by one head ----
            emit_loads(0)
            emit_loads(1)
            # v tiles: split load by s-chunk so attn of head 0 starts early
            for sc in range(8):
                nc.sync.dma_start(out=vS_t[:, sc, :, :], in_=vS_d[:, sc, :, :])
            nc.sync.dma_start(out=WT_t[:], in_=WT_d)

            for h in range(H):
                if h + 2 < H:
                    emit_loads(h + 2)
                if h >= 1:
                    emit_post_pe(h - 1)
                    emit_post_dve(h - 1)
                emit_scores(h)
            emit_post_pe(H - 1)
            emit_post_dve(H - 1)

            # ---- inv = 1/sqrt(mean+eps), out_proj, scale, store ----
            ss_ps = [psz.tile([128, 1], F32, tag="z", name=f"ss{tb}")
                     for tb in range(2)]
            for tb in range(2):
                nc.tensor.matmul(ss_ps[tb][:],
                                 sqacc_t[:, tb * 128:(tb + 1) * 128],
                                 ones96_t[:], start=True, stop=True)
            inv_t = []
            for tb in range(2):
                tmp_t = wp.tile([128, 1], F32, tag=f"tmp{tb}")
                nc.scalar.activation(tmp_t[:], ss_ps[tb][:], AF.Sqrt,
                                     scale=1.0 / HID, bias=eps_t[:])
                iv = wp.tile([128, 1], F32, tag=f"inv{tb}")
                nc.vector.reciprocal(iv[:], tmp_t[:])
                inv_t.append(iv)

            for p in range(P):
                for tb in range(2):
                    o_ps = psw.tile([128, HID], F32, tag="w",
                                    name=f"o_{p}_{tb}")
                    for ci in range(4):
                        nc.tensor.matmul(o_ps[:],
                                         X_t[:, p, ci, tb * 128:(tb + 1) * 128],
                                         WT_t[:, ci, :],
                                         start=(ci == 0), stop=(ci == 3))
                    o_sb = wp.tile([128, HID], BF16, tag="osb")
                    nc.vector.tensor_scalar_mul(o_sb[:], o_ps[:], inv_t[tb][:])
                    nc.sync.dma_start(out=out_d[tb * 128:(tb + 1) * 128, p, :],
                                      in_=o_sb[:])

    nc.compile()
    return nc


def _get_program():
    if "nc" not in _prog_cache:
        _prog_cache["nc"] = _build_program()
    return _prog_cache["nc"]


def _prepare_in_maps(q, k, v, attn_bias, key_padding_mask, outcell_index,
                     local_attention_weight, expand_mask, out_proj_weight,
                     attn_ln_weight):
    q = np.asarray(q, dtype=np.float32)
    k = np.asarray(k, dtype=np.float32)
    v = np.asarray(v, dtype=np.float32)
    attn_bias = np.asarray(attn_bias, dtype=np.float32)
    kpm = np.asarray(key_padding_mask)
    idx = np.asarray(outcell_index).astype(np.int64)
    law = np.asarray(local_attention_weight, dtype=np.float32)
    emask = np.asarray(expand_mask)
    W = np.asarray(out_proj_weight, dtype=np.float32)
    lnw = np.asarray(attn_ln_weight, dtype=np.float32)

    WT = np.ascontiguousarray((W * lnw[None, :]).T)  # [hid, o], ln folded
    eye_np = np.eye(128, dtype=ml_dtypes.bfloat16)
    ones128_np = np.ones((128, 1), dtype=ml_dtypes.bfloat16)
    ones96_np = np.ones((D, 1), dtype=np.float32)

    in_maps = []
    for c in range(8):
        b, th = c // 2, c % 2
        tsl = slice(th * TQ, (th + 1) * TQ)

        # kT [H, 96, S]: kf[s, p, h*32+hd] with s-expansion host-gathered
        kf = np.concatenate([k[b], k[b][idx[b]]], axis=0)  # [S, P, HID]
        kT = kf.reshape(S, P, H, HD).transpose(2, 1, 3, 0).reshape(H, D, S)
        qT = q[b, tsl].reshape(TQ, P, H, HD).transpose(2, 1, 3, 0) \
            .reshape(H, D, TQ)

        # vS [128, 8, H, 96]: vS[part, sc, h, (p,hd)] = vf[sc*128+part, ...]
        vf = np.concatenate([v[b], v[b][idx[b]]], axis=0)  # [S, P, HID]
        vS = vf.reshape(8, 128, P, H, HD).transpose(1, 0, 3, 2, 4) \
            .reshape(128, 8, H, D)

        # masked bias [H, 256, S]
        bias_c = np.ascontiguousarray(attn_bias[b, :, tsl, :])
        kpmS = np.concatenate([kpm[b], emask[b]])           # [S]
        if kpmS.any():
            bias_c[:, :, kpmS] = NEG
        cut = law[b, tsl] <= CUTOFF                         # [256, S]
        if cut.any():
            bias_c[:, cut] = NEG
        # transpose to [H, S, 256] -> [H, 128, 8, 256]
        btT = bias_c.transpose(0, 2, 1).reshape(H, 8, 128, TQ) \
            .transpose(0, 2, 1, 3)                          # [H, 128, 8, TQ]
        btT = btT.reshape(H, 128, 2, 4 * TQ)
        bt = np.empty((H, 128, 2, 4 * TQ), dtype=ml_dtypes.bfloat16)
        bt[:N_EB] = np.exp(btT[:N_EB])
        bt[N_EB:] = btT[N_EB:]

        lawT = law[b, tsl].T.reshape(8, 128, TQ).transpose(1, 0, 2) \
            .reshape(128, 2, 4 * TQ)

        in_maps.append(dict(
            qT=qT.astype(np.float16),
            kT=np.ascontiguousarray(kT).astype(np.float16),
            vS=np.ascontiguousarray(vS).astype(ml_dtypes.bfloat16),
            bt=np.ascontiguousarray(bt),
            lawT=np.ascontiguousarray(lawT).astype(ml_dtypes.bfloat16),
            WT=WT.reshape(4, 128, HID).transpose(1, 0, 2).astype(
                ml_dtypes.bfloat16).copy(),
            eye128=eye_np,
            ones128=ones128_np,
            ones96=ones96_np,
        ))
    return in_maps


def kernel(**inputs):
    in_maps = _prepare_in_maps(**inputs)
    nc = _get_program()
    res = run_bass_kernel_spmd(nc, in_maps, list(range(8)))

    out = np.empty((B, T, P, HID), dtype=np.float32)
    for c in range(8):
        b, th = c // 2, c % 2
        out[b, th * TQ:(th + 1) * TQ] = res.results[c]["out"].astype(np.float32)
    return out
